# revision 23
# baseline (speedup 1.0000x reference)
"""Paged decode attention (nn_Attention_5626407157951) on 8 Trainium2 cores.

Tensor-parallel over heads: each core owns 4 of 32 heads. Per core:
  qkv = hidden @ W_pack[:, own cols]      (bf16 matmuls, fp32 acc)
  rotary(q, k) at pos=hist                (DVE, fp32; host-built cos/sin)
  scores_T[s, (h,pair)] = K_pair^T q      (PE, K stationary bf16, q moving)
  softmax without max-subtraction; new token handled analytically:
      out = (sum_s exp(s)*v_s + e_new*v_new) / (sum_s exp(s) + e_new)
  out_partial = attn @ o_proj[:, own dims].T ; host sums the 8 partials.

HBM-traffic-shaped v2 (trace-driven):
  - The whole per-core KV working set is packed host-side into ONE flat
    [128, L] bf16 stream in slot order: per request, 4 head-major K^T
    segments of EXACTLY hist tokens (no 128-rounding; score matmuls for
    the ragged tail read a few elements of the neighbouring segment,
    which is finite bf16 data and gets zeroed by the zmask), then pb
    V blocks of [128 s, 4*128 d].  The stream is DMA'd in ~11 request-
    aligned chunks of <=24KiB per partition line: every packet is large,
    the DMA engines never see the tiny per-request transfers that capped
    throughput at the tail of the old stream.
  - W_pack is re-laid out to [128, KT*1536] so it loads in 8 big DMAs
    (12KiB lines) instead of 32 small ones.
  - The epilogue is split: slots 0..15 run their (attn+e*v_new)*rec
    scaling + o_proj right after slot 15's attnV, overlapped with the
    KV stream of slots 16..31; only the second half remains in the tail.
    e_new is pre-broadcast once into vTe = e*vT, so each half needs only
    one rec broadcast.
  - Output partials are returned in bf16 (host sums in fp32).
"""

import math
import os

import ml_dtypes
import numpy as np

import concourse.bass as bass
import concourse.mybir as mybir
import concourse.tile as tile
from concourse.bass_utils import run_bass_kernel_spmd
from concourse.vector_clock import ScopedClock

B = 32          # batch (decode requests)
H = 32          # total heads
HL = 4          # heads per core
D = 128         # head dim
HID = 4096
BS = 64         # cache block size
NBLK = 16       # blocks per request
MAX_KV = NBLK * BS
NCORES = 8
KT = HID // 128         # 32 contraction tiles for qkv proj
ROPE_BASE = 10000.0
CP_MAX = 8192           # max packed-KV chunk length (elems per partition)
M2 = 16                 # slots per epilogue half

F32 = mybir.dt.float32
BF = mybir.dt.bfloat16
BF_NP = ml_dtypes.bfloat16
EXP_FN = mybir.ActivationFunctionType.Exp
COPY_FN = mybir.ActivationFunctionType.Copy
MUL = mybir.AluOpType.mult
ADD = mybir.AluOpType.add
SUB = mybir.AluOpType.subtract
DIV = mybir.AluOpType.divide

LAST_RESULTS = None  # test harness peeks at this for profiling info

# ---------------------------------------------------------------------------
# This walrus build accepts very few sync-waits per instruction; the Tile
# kernel-tail drain accumulates one wait per sem lane. Split the waits over
# several drain instructions (all before the barrier, so semantics hold).
_MAX_DRAIN_WAITS = 1


def _patched_drain_and_barrier(self, tick_clock, wait_clock):
    nc = self.nc
    drain_inst = nc.sync.drain()
    wait_clock.add_sem_waits(
        drain_inst.ins, ScopedClock({None: tick_clock.global_clock})
    )
    si = drain_inst.ins.sync_info
    if si is not None and si.on_wait and len(si.on_wait) > _MAX_DRAIN_WAITS:
        waits = list(si.on_wait)
        drain_inst.ins.sync_info = mybir.SyncInfo(
            on_wait=waits[:_MAX_DRAIN_WAITS], on_update=list(si.on_update or [])
        )
        rest = waits[_MAX_DRAIN_WAITS:]
        for i in range(0, len(rest), _MAX_DRAIN_WAITS):
            extra = nc.sync.drain()
            extra.ins.sync_info = mybir.SyncInfo(
                on_wait=rest[i : i + _MAX_DRAIN_WAITS], on_update=[]
            )
    nc.all_engine_barrier()
    popped = nc._tile_sem_poison_stack.pop()
    assert popped is self._sem_poison
    nc.clear_and_free_semaphores(list(self.sems.allocated().values()))
    nc.all_engine_barrier()


tile.TileContext._drain_and_barrier = _patched_drain_and_barrier


def _split_excess_waits(nc, limit=1):
    """Walrus rejects instructions carrying more than ~1 sync wait. Hoist the
    excess onto NoOps inserted just before, on the same engine queue (the
    queue blocks on them first, so semantics are identical)."""
    for fn in nc.m.functions:
        for bb in fn.blocks:
            out = []
            changed = False
            for inst in list(bb.instructions):
                si = getattr(inst, "sync_info", None)
                if si is not None and si.on_wait and len(si.on_wait) > limit:
                    waits = list(si.on_wait)
                    extra, keep = waits[:-limit], waits[-limit:]
                    for i in range(0, len(extra), limit):
                        nop = mybir.InstNoOp(
                            name=nc.get_next_instruction_name(),
                            ins=[], outs=[], engine=inst.engine,
                            sync_info=mybir.SyncInfo(
                                on_wait=extra[i : i + limit], on_update=[]
                            ),
                        )
                        nc.register_instruction(nop)
                        out.append(nop)
                    inst.sync_info = mybir.SyncInfo(
                        on_wait=keep, on_update=list(si.on_update or [])
                    )
                    changed = True
                out.append(inst)
            if changed:
                bb.instructions = out
# ---------------------------------------------------------------------------


def _build_nc(slots, chunks, total_l):
    """Build the SPMD bass module. `slots[i]` holds the geometry of the
    request processed in slot i (descending pairs); `chunks` is the list of
    (offset, length) windows of the packed KV stream; `total_l` its length.
    The `ident` input is a permutation matrix mapping qkv rows (original b)
    to slot columns; the host unpermutes the output rows."""
    nc = bass.Bass()

    def param(name, shape, dt):
        return nc.declare_dram_parameter(name, list(shape), dt, isOutput=False)

    hT = param("hT", [128, KT, B], BF)
    wpp = param("wpp", [128, KT * 3 * HL * D], BF)
    wo = param("wo", [HL, 128, HID], BF)
    kvs = param("kvs", [128, total_l], BF)
    cs = param("cs", [B, 4 * HL * D], F32)
    zmaskp = param("zmask", [128, B], F32)           # 1 iff row s < hist (tail pair)
    identp = param("ident", [B, B], BF)              # permutation matrix
    out_part = nc.declare_dram_parameter("out_part", [B, HID], BF, isOutput=True)

    HD = HL * D  # 512 local attention dims

    with tile.TileContext(nc) as tc:
        with (
            tc.tile_pool(name="const", bufs=1) as cpool,
            tc.tile_pool(name="work", bufs=1) as wpool,
            tc.tile_pool(name="wtiles", bufs=3) as wtp,
            tc.tile_pool(name="wop", bufs=4) as wop,
            tc.tile_pool(name="kv", bufs=6) as kvp,
            tc.tile_pool(name="small", bufs=3) as smp,
        ):
            # ---- constants ----
            ident = cpool.tile([B, B], BF)
            nc.sync.dma_start(out=ident[:], in_=identp[:])
            ones = cpool.tile([128, 1], BF)
            nc.vector.memset(ones[:], 1.0)
            onesb = cpool.tile([1, HL * B], BF)
            nc.vector.memset(onesb[:], 1.0)
            zmask = cpool.tile([128, B], F32)
            nc.sync.dma_start(out=zmask[:], in_=zmaskp[:])
            cs_sb = cpool.tile([B, 4 * HD], F32)
            nc.sync.dma_start(out=cs_sb[:], in_=cs[:])
            hT_sb = cpool.tile([128, KT, B], BF)
            nc.sync.dma_start(out=hT_sb[:], in_=hT[:])

            # packed-KV chunk loads (one DMA each, huge per-partition lines)
            kv_tiles = {}

            def load_chunk(c):
                off, ln = chunks[c]
                kvt = kvp.tile([128, CP_MAX], BF, tag="kv")
                nc.sync.dma_start(out=kvt[:, 0:ln], in_=kvs[:, off : off + ln])
                kv_tiles[c] = kvt

            # accumulators written per-slot, read by the epilogue halves
            atsb = wpool.tile([128, HL * B], F32)   # cached attn, col h*32+i
            nc.vector.memset(atsb[:], 0.0)
            dnm = wpool.tile([1, HL * B], F32)      # cached denom, col h*32+i
            nc.vector.memset(dnm[:], 0.0)

            wo_tiles = {}

            def issue_wo(i):
                wot = wop.tile([128, HID], BF, tag="wot")
                nc.sync.dma_start(out=wot[:], in_=wo[i])
                wo_tiles[i] = wot

            with tc.tile_pool(name="psA", bufs=1, space="PSUM") as psA:
                # PE warmup transpose so `ident` is observed by PE before the
                # real (fp32, single-wait-slot) transposes below.
                tp0 = psA.tile([B, B], BF, tag="tp0")
                nc.tensor.transpose(tp0[:], ident[:], ident[:])

                # ---- phase 1: qkv = hidden @ W_pack (bf16, 8 big weight DMAs) ----
                qkv_ps = psA.tile([B, 3 * HD], F32, tag="qkv")
                for wc in range(8):
                    wpt = wtp.tile([128, 4 * 3 * HD], BF, tag="wpt")
                    nc.sync.dma_start(
                        out=wpt[:],
                        in_=wpp[:, wc * 4 * 3 * HD : (wc + 1) * 4 * 3 * HD],
                    )
                    for j in range(4):
                        kt = wc * 4 + j
                        for n in range(3):
                            nc.tensor.matmul(
                                qkv_ps[:, n * HD : (n + 1) * HD],
                                hT_sb[:, kt, :],
                                wpt[:, j * 3 * HD + n * HD : j * 3 * HD + (n + 1) * HD],
                                start=(kt == 0),
                                stop=(kt == KT - 1),
                            )

                # KV chunk preloads + o_proj weights queue behind the wp
                # stream on the sync DMA queue, in consumption order.
                for c in range(min(6, len(chunks))):
                    load_chunk(c)
                for i in range(HL):
                    issue_wo(i)

                # ---- phase 2: rotary (fp32 DVE, reading PSUM directly),
                # rotated q/k written straight to the bf16 staging tile ----
                qkv_bf = wpool.tile([B, 3 * HD], BF)
                nc.scalar.copy(qkv_bf[:, 2 * HD :], qkv_ps[:, 2 * HD :])

                def rope(src_off, cs_off):
                    src = qkv_ps[:, src_off : src_off + HD]
                    t1 = wpool.tile([B, HD], F32, tag="rope_t1")
                    nc.vector.tensor_tensor(
                        t1[:], src, cs_sb[:, cs_off : cs_off + HD], MUL
                    )
                    sh = wpool.tile([B, HD], F32, tag="rope_sh")
                    sh4 = sh[:].rearrange("b (h d) -> b h d", h=HL)
                    sr4 = src.rearrange("b (h d) -> b h d", h=HL)
                    nc.scalar.copy(sh4[:, :, 0:64], sr4[:, :, 64:128])
                    nc.scalar.copy(sh4[:, :, 64:128], sr4[:, :, 0:64])
                    nc.vector.tensor_tensor(
                        sh[:], sh[:], cs_sb[:, cs_off + HD : cs_off + 2 * HD], MUL
                    )
                    nc.vector.tensor_tensor(
                        qkv_bf[:, src_off : src_off + HD], t1[:], sh[:], ADD
                    )

                rope(0, 0)
                rope(HD, 2 * HD)

            # PE transposes (bf16, permuted to slot order by `ident`)
            qT_bf = wpool.tile([128, HL * B], BF)
            vT = wpool.tile([128, HL * B], F32)
            prod = wpool.tile([128, HL * B], BF)
            e_new = wpool.tile([1, HL * B], F32)
            vTe = wpool.tile([128, HL * B], F32)    # e_new * v_new, broadcast
            with tc.tile_pool(name="psT", bufs=2, space="PSUM") as psT:
                for h in range(HL):
                    tpq = psT.tile([128, B], BF, tag="tpq")
                    nc.tensor.transpose(
                        tpq[:], qkv_bf[:, h * D : (h + 1) * D], ident[:]
                    )
                    tpk = psT.tile([128, B], BF, tag="tpk")
                    nc.tensor.transpose(
                        tpk[:], qkv_bf[:, HD + h * D : HD + (h + 1) * D], ident[:]
                    )
                    tpv = psT.tile([128, B], BF, tag="tpv")
                    nc.tensor.transpose(
                        tpv[:], qkv_bf[:, 2 * HD + h * D : 2 * HD + (h + 1) * D],
                        ident[:],
                    )
                    nc.vector.tensor_copy(qT_bf[:, h * B : (h + 1) * B], tpq[:])
                    nc.scalar.copy(vT[:, h * B : (h + 1) * B], tpv[:])
                    # new-token score terms: q_d * k_d (slot order), bf16
                    nc.vector.tensor_tensor(
                        prod[:, h * B : (h + 1) * B],
                        qT_bf[:, h * B : (h + 1) * B], tpk[:], MUL
                    )

                sn_ps = psT.tile([1, HL * B], F32, tag="sn", bufs=1)
                nc.tensor.matmul(sn_ps[:], ones[:], prod[:], start=True, stop=True)
                nc.scalar.activation(e_new[:], sn_ps[:], EXP_FN)

                # broadcast e_new over partitions once; fold into vT
                e_bfx = wpool.tile([1, HL * B], BF)
                nc.scalar.copy(e_bfx[:], e_new[:])
                ebp = psT.tile([128, HL * B], F32, tag="ebp", bufs=1)
                nc.tensor.matmul(ebp[:], onesb[:], e_bfx[:], start=True, stop=True)
                nc.vector.tensor_tensor(vTe[:], vT[:], ebp[:], MUL)

            dtot = wpool.tile([1, HL * B], F32)
            rec = wpool.tile([1, HL * B], F32)
            nc.vector.memset(rec[:], 1.0)

            atsb_v = atsb[:].rearrange("d (h b2) -> d h b2", h=HL)
            vTe_v = vTe[:].rearrange("d (h b2) -> d h b2", h=HL)
            dnm_v = dnm[:].rearrange("o (h b2) -> o h b2", h=HL)
            e_v = e_new[:].rearrange("o (h b2) -> o h b2", h=HL)
            dtot_v = dtot[:].rearrange("o (h b2) -> o h b2", h=HL)
            rec_v = rec[:].rearrange("o (h b2) -> o h b2", h=HL)

            def ep_math_half(lo):
                # attn = (atsb + e_new*vT) * rec for slot columns lo..lo+15
                r_h = wpool.tile([1, HL * M2], F32, tag=f"rh{lo}")
                nc.vector.tensor_copy(
                    r_h[:].rearrange("o (h m) -> o h m", h=HL),
                    rec_v[:, :, lo : lo + M2],
                )
                r_bf = wpool.tile([1, HL * M2], BF, tag=f"rbf{lo}")
                nc.vector.tensor_copy(r_bf[:], r_h[:])
                att = wpool.tile([128, HL * M2], F32, tag=f"att{lo}")
                av = att[:].rearrange("d (h m) -> d h m", h=HL)
                with tc.tile_pool(name=f"psE{lo}", bufs=1, space="PSUM") as psE:
                    rbp = psE.tile([128, HL * M2], F32, tag="rbp")
                    nc.tensor.matmul(rbp[:], onesb[:], r_bf[:], start=True, stop=True)
                    nc.vector.tensor_tensor(
                        av, atsb_v[:, :, lo : lo + M2], vTe_v[:, :, lo : lo + M2], ADD
                    )
                    nc.vector.tensor_tensor(att[:], att[:], rbp[:], MUL)
                at_bf = wpool.tile([128, HL * M2], BF, tag=f"atb{lo}")
                nc.vector.tensor_copy(at_bf[:], att[:])
                return at_bf

            def oproj_chunk(at_bf, outf, n, psO):
                opsn = psO.tile([M2, HD], F32, tag="ops")
                for h in range(HL):
                    nc.tensor.matmul(
                        opsn[:],
                        at_bf[:, h * M2 : (h + 1) * M2],
                        wo_tiles[h][:, n * HD : (n + 1) * HD],
                        start=(h == 0),
                        stop=(h == HL - 1),
                    )
                nc.vector.tensor_copy(outf[:, n * HD : (n + 1) * HD], opsn[:])

            # ---- phase 3: per-request paged attention (slot order) ----
            # Software-pipelined one request ahead: scores(i+1) is issued
            # before attnV(i) so the PE never stalls on the mask->exp->cast
            # round trip through DVE/ACT.
            ph_tiles = {}
            with (
                tc.tile_pool(name="psB", bufs=2, space="PSUM") as psB,
                tc.tile_pool(name="psB2", bufs=2, space="PSUM") as psB2,
                tc.tile_pool(name="psO", bufs=1, space="PSUM") as psO,
            ):
                def ensure_chunk(i):
                    c = slots[i]["chunk"]
                    if c not in kv_tiles:
                        load_chunk(c)

                def do_scores(i):
                    m = slots[i]
                    pb = m["pb"]
                    if pb == 0:
                        return
                    kvt = kv_tiles[m["chunk"]]
                    rel, hist = m["rel"], m["hist"]
                    # scores^T: [128(s), (h, pair)]
                    scp = psB.tile([128, HL, pb], F32, tag="scp")
                    for h in range(HL):
                        qh = qT_bf[:, h * B + i : h * B + i + 1]
                        base = rel + h * hist
                        for p in range(pb):
                            nc.tensor.matmul(
                                scp[:, h, p : p + 1],
                                kvt[:, base + p * 128 : base + p * 128 + 128],
                                qh, start=True, stop=True,
                            )
                    # probs = exp(scores) in bf16 straight off PSUM; rows
                    # >= hist in the tail pair are zeroed by an ACT copy with
                    # a per-partition 0/1 scale (same engine, no extra hop)
                    ph = smp.tile([128, HL, pb], BF, tag="ph", bufs=9)
                    nc.scalar.activation(ph[:], scp[:], EXP_FN)
                    if m["rtail"] < 128:
                        nc.scalar.activation(
                            ph[:, :, pb - 1], ph[:, :, pb - 1], COPY_FN,
                            scale=zmask[:, i : i + 1],
                        )
                    ph_tiles[i] = ph

                def do_attnv(i):
                    m = slots[i]
                    pb = m["pb"]
                    if pb > 0:
                        kvt = kv_tiles[m["chunk"]]
                        ph = ph_tiles.pop(i)
                        vbase = m["rel"] + HL * m["hist"]
                        # attn^T[d, h] = sum_s p[s] * V[s, d]
                        atp = psB.tile([128, HL], F32, tag="atp")
                        for h in range(HL):
                            for p in range(pb):
                                o = vbase + p * HD + h * D
                                nc.tensor.matmul(
                                    atp[:, h : h + 1], kvt[:, o : o + D],
                                    ph[:, h, p : p + 1],
                                    start=(p == 0), stop=(p == pb - 1),
                                )
                        # on vector, NOT scalar: a scalar-queue copy waiting on
                        # attnv(i) would make exp(i+1) wait behind it, coupling
                        # the in-order PE and ACT queues into a per-slot
                        # round-trip chain that paces the whole stream.
                        nc.vector.tensor_copy(atsb_v[:, :, i], atp[:])
                        # denominators: column sums of probs
                        dsp = psB2.tile([1, HL * pb], F32, tag="dsp")
                        nc.tensor.matmul(
                            dsp[:], ones[:],
                            ph[:].rearrange("s h p -> s (h p)"),
                            start=True, stop=True,
                        )
                        nc.vector.reduce_sum(
                            dnm_v[:, :, i],
                            dsp[:].rearrange("o (h p) -> o h p", h=HL),
                            axis=mybir.AxisListType.X,
                        )
                    # incremental 1/(denom + e_new) for this slot
                    nc.vector.tensor_tensor(
                        dtot_v[:, :, i], dnm_v[:, :, i], e_v[:, :, i], ADD
                    )
                    nc.vector.reciprocal(rec_v[:, :, i], dtot_v[:, :, i])

                # scores run LAG slots ahead of attnV: by the time attnv(i)
                # reaches the in-order PE queue, exp(i) is long finished, so
                # the PE never blocks on the ACT round trip mid-stream.
                LAG = 6
                outf1 = wpool.tile([M2, HID], BF)
                for j in range(LAG):
                    ensure_chunk(j)
                    do_scores(j)
                at1 = None
                for i in range(B):
                    do_attnv(i)
                    if i + LAG < B:
                        ensure_chunk(i + LAG)
                        do_scores(i + LAG)
                    if i == M2 - 1:
                        at1 = ep_math_half(0)
                    if M2 <= i < M2 + 8:
                        oproj_chunk(at1, outf1, i - M2, psO)
                    if i == M2 + 8:
                        nc.gpsimd.dma_start(out=out_part[0:M2, :], in_=outf1[:])

            # ---- epilogue half 2 (after loop pools close) ----
            at2 = ep_math_half(M2)
            outf2 = wpool.tile([M2, HID], BF)
            with tc.tile_pool(name="psO2", bufs=2, space="PSUM") as psO2:
                for n in range(8):
                    oproj_chunk(at2, outf2, n, psO2)
            nc.gpsimd.dma_start(out=out_part[M2 : 2 * M2, :], in_=outf2[:])

    _split_excess_waits(nc)
    return nc


def _host_prep(hidden, W_pack, o_proj_weight, k_cache, v_cache, hist, block_offsets):
    """Build the 8 per-core input maps (numpy only)."""
    hidden = np.asarray(hidden, np.float32)
    W_pack = np.asarray(W_pack, np.float32)
    o_proj_weight = np.asarray(o_proj_weight, np.float32)
    k_cache = np.asarray(k_cache, np.float32)
    v_cache = np.asarray(v_cache, np.float32)
    hist = np.asarray(hist, np.int64)
    block_offsets = np.asarray(block_offsets, np.int64)

    pairs = [int((h + 127) // 128) for h in hist]
    # slot order: descending pairs so the first epilogue half (slots 0..15)
    # triggers while plenty of KV stream remains to overlap its o_proj
    order = sorted(range(B), key=lambda b: (-pairs[b], b))

    # rope tables, scale folded into the q tables
    inv_freq = 1.0 / (ROPE_BASE ** (np.arange(0, D, 2, dtype=np.float32) / D))
    ang = hist.astype(np.float32)[:, None] * inv_freq[None, :]        # [B, 64]
    cos128 = np.concatenate([np.cos(ang), np.cos(ang)], -1)           # [B, 128]
    sin128 = np.concatenate([np.sin(ang), np.sin(ang)], -1)
    sign = np.concatenate([-np.ones(64), np.ones(64)]).astype(np.float32)
    sc = 1.0 / math.sqrt(D)
    tile_h = lambda x: np.tile(x, (1, HL)).astype(np.float32)         # [B, 512]
    cs = np.concatenate(
        [tile_h(cos128 * sc), tile_h(sin128 * sign * sc),
         tile_h(cos128), tile_h(sin128 * sign)], -1,
    )                                                                 # [B, 2048]

    # per-slot geometry + packed-stream layout
    hs = [int(hist[order[i]]) for i in range(B)]
    pbs = [pairs[order[i]] for i in range(B)]
    rtail = [hs[i] - 128 * (pbs[i] - 1) if pbs[i] > 0 else 128 for i in range(B)]
    Ls = [HL * hs[i] + pbs[i] * HL * D for i in range(B)]

    chunks = []
    chunk_of_slot = []
    G = []
    cur_off = cur_len = 0
    for i in range(B):
        if cur_len + Ls[i] > CP_MAX:
            chunks.append((cur_off, cur_len))
            cur_off += cur_len
            cur_len = 0
        G.append(cur_off + cur_len)
        chunk_of_slot.append(len(chunks))
        cur_len += Ls[i]
    chunks.append((cur_off, cur_len))
    total_l = cur_off + cur_len
    slots = [
        dict(hist=hs[i], pb=pbs[i], rtail=rtail[i],
             rel=G[i] - chunks[chunk_of_slot[i]][0], chunk=chunk_of_slot[i])
        for i in range(B)
    ]

    zmask = np.zeros((128, B), dtype=np.float32)
    for i, r in enumerate(rtail):
        zmask[:r, i] = 1.0

    hT = np.ascontiguousarray(hidden.T)                               # [4096, 32]
    hT_bf = np.ascontiguousarray(
        hT.astype(BF_NP).reshape(KT, 128, B).transpose(1, 0, 2)
    )                                                                 # [128, KT, B]

    # gather caches via the block table (b-major)
    kf = k_cache[block_offsets.reshape(-1)].reshape(B, MAX_KV, H, D)
    vf = v_cache[block_offsets.reshape(-1)].reshape(B, MAX_KV, H, D)

    # permutation matrix: column slot i picks original request order[i]
    ident = np.zeros((B, B), dtype=BF_NP)
    ident[np.asarray(order), np.arange(B)] = 1.0

    in_maps = []
    for c in range(NCORES):
        h0 = c * HL
        qcols = np.arange(h0 * D, (h0 + HL) * D)
        wp_c = np.concatenate(
            [W_pack[:, qcols], W_pack[:, HID + qcols], W_pack[:, 2 * HID + qcols]],
            axis=1,
        )                                                             # [4096, 1536]
        wp_bf = np.ascontiguousarray(
            wp_c.astype(BF_NP).reshape(KT, 128, 3 * HL * D)
            .transpose(1, 0, 2).reshape(128, KT * 3 * HL * D)
        )                                                             # [128, KT*1536]

        wo_c = np.ascontiguousarray(o_proj_weight[:, qcols].T)        # [512, 4096]
        wo_bf = wo_c.astype(BF_NP).reshape(HL, 128, HID)

        kvs = np.zeros((128, total_l), dtype=BF_NP)
        for i in range(B):
            b = order[i]
            hb, pb, g = hs[i], pbs[i], G[i]
            if pb == 0:
                continue
            kb = kf[b, :hb, h0 : h0 + HL, :]                          # [hb, 4, 128]
            kvs[:, g : g + HL * hb] = (
                kb.transpose(2, 1, 0).reshape(D, HL * hb).astype(BF_NP)
            )
            vb = np.zeros((pb * 128, HL, D), np.float32)
            vb[:hb] = vf[b, :hb, h0 : h0 + HL, :]
            g4 = g + HL * hb
            kvs[:, g4 : g4 + pb * HL * D] = (
                vb.reshape(pb, 128, HL * D).transpose(1, 0, 2)
                .reshape(128, pb * HL * D).astype(BF_NP)
            )

        in_maps.append({
            "hT": hT_bf, "wpp": wp_bf, "wo": wo_bf,
            "kvs": kvs,
            "cs": cs, "zmask": zmask, "ident": ident,
        })
    return slots, chunks, total_l, order, in_maps


def kernel(hidden_states, W_pack, o_proj_weight, k_cache, v_cache,
           history_lengths, block_offsets):
    global LAST_RESULTS
    slots, chunks, total_l, order, in_maps = _host_prep(
        hidden_states, W_pack, o_proj_weight, k_cache, v_cache,
        history_lengths, block_offsets,
    )
    nc = _build_nc(slots, chunks, total_l)
    trace = bool(int(os.environ.get("KERNEL_TRACE", "0")))
    res = run_bass_kernel_spmd(nc, in_maps, list(range(NCORES)), trace=trace)
    LAST_RESULTS = res
    acc = np.zeros((B, HID), np.float32)
    for c in range(NCORES):
        acc += res.results[c]["out_part"].astype(np.float32)
    out = np.zeros((B, HID), np.float32)
    out[np.asarray(order)] = acc                   # slot rows -> original rows
    return out


# revision 26
# speedup vs baseline: 1.1524x; 1.1524x over previous
"""Paged decode attention (nn_Attention_5626407157951) on 8 Trainium2 cores.

Tensor-parallel over heads: each core owns 4 of 32 heads. Per core:
  qkv = hidden @ W_pack[:, own cols]      (bf16 matmuls, fp32 acc)
  rotary(q, k) at pos=hist                (DVE, fp32; host-built cos/sin)
  scores_T[s, (h,pair)] = K_pair^T q      (PE, K stationary bf16, q moving)
  softmax without max-subtraction; new token handled analytically:
      out = (sum_s exp(s)*v_s + e_new*v_new) / (sum_s exp(s) + e_new)
  out_partial = attn @ o_proj[:, own dims].T ; host sums the 8 partials.

HBM-traffic-shaped v2 (trace-driven):
  - The whole per-core KV working set is packed host-side into ONE flat
    [128, L] bf16 stream in slot order: per request, 4 head-major K^T
    segments of EXACTLY hist tokens (no 128-rounding; score matmuls for
    the ragged tail read a few elements of the neighbouring segment,
    which is finite bf16 data and gets zeroed by the zmask), then pb
    V blocks of [128 s, 4*128 d].  The stream is DMA'd in ~11 request-
    aligned chunks of <=24KiB per partition line: every packet is large,
    the DMA engines never see the tiny per-request transfers that capped
    throughput at the tail of the old stream.
  - W_pack is re-laid out to [128, KT*1536] so it loads in 8 big DMAs
    (12KiB lines) instead of 32 small ones.
  - The epilogue is split: slots 0..15 run their (attn+e*v_new)*rec
    scaling + o_proj right after slot 15's attnV, overlapped with the
    KV stream of slots 16..31; only the second half remains in the tail.
    e_new is pre-broadcast once into vTe = e*vT, so each half needs only
    one rec broadcast.
  - Output partials are returned in bf16 (host sums in fp32).
"""

import math
import os

import ml_dtypes
import numpy as np

import concourse.bass as bass
import concourse.mybir as mybir
import concourse.tile as tile
from concourse.bass_utils import run_bass_kernel_spmd
from concourse.vector_clock import ScopedClock

B = 32          # batch (decode requests)
H = 32          # total heads
HL = 4          # heads per core
D = 128         # head dim
HID = 4096
BS = 64         # cache block size
NBLK = 16       # blocks per request
MAX_KV = NBLK * BS
NCORES = 8
KT = HID // 128         # 32 contraction tiles for qkv proj
ROPE_BASE = 10000.0
CP_MAX = 8192           # max packed-KV chunk length (elems per partition)
M2 = 16                 # slots per epilogue half

F32 = mybir.dt.float32
BF = mybir.dt.bfloat16
BF_NP = ml_dtypes.bfloat16
EXP_FN = mybir.ActivationFunctionType.Exp
COPY_FN = mybir.ActivationFunctionType.Copy
MUL = mybir.AluOpType.mult
ADD = mybir.AluOpType.add
SUB = mybir.AluOpType.subtract
DIV = mybir.AluOpType.divide

LAST_RESULTS = None  # test harness peeks at this for profiling info

# ---------------------------------------------------------------------------
# This walrus build accepts very few sync-waits per instruction; the Tile
# kernel-tail drain accumulates one wait per sem lane. Split the waits over
# several drain instructions (all before the barrier, so semantics hold).
_MAX_DRAIN_WAITS = 1


def _patched_drain_and_barrier(self, tick_clock, wait_clock):
    nc = self.nc
    drain_inst = nc.sync.drain()
    wait_clock.add_sem_waits(
        drain_inst.ins, ScopedClock({None: tick_clock.global_clock})
    )
    si = drain_inst.ins.sync_info
    if si is not None and si.on_wait and len(si.on_wait) > _MAX_DRAIN_WAITS:
        waits = list(si.on_wait)
        drain_inst.ins.sync_info = mybir.SyncInfo(
            on_wait=waits[:_MAX_DRAIN_WAITS], on_update=list(si.on_update or [])
        )
        rest = waits[_MAX_DRAIN_WAITS:]
        for i in range(0, len(rest), _MAX_DRAIN_WAITS):
            extra = nc.sync.drain()
            extra.ins.sync_info = mybir.SyncInfo(
                on_wait=rest[i : i + _MAX_DRAIN_WAITS], on_update=[]
            )
    nc.all_engine_barrier()
    popped = nc._tile_sem_poison_stack.pop()
    assert popped is self._sem_poison
    nc.clear_and_free_semaphores(list(self.sems.allocated().values()))
    nc.all_engine_barrier()


tile.TileContext._drain_and_barrier = _patched_drain_and_barrier


def _split_excess_waits(nc, limit=1):
    """Walrus rejects instructions carrying more than ~1 sync wait. Hoist the
    excess onto NoOps inserted just before, on the same engine queue (the
    queue blocks on them first, so semantics are identical)."""
    for fn in nc.m.functions:
        for bb in fn.blocks:
            out = []
            changed = False
            for inst in list(bb.instructions):
                si = getattr(inst, "sync_info", None)
                if si is not None and si.on_wait and len(si.on_wait) > limit:
                    waits = list(si.on_wait)
                    extra, keep = waits[:-limit], waits[-limit:]
                    for i in range(0, len(extra), limit):
                        nop = mybir.InstNoOp(
                            name=nc.get_next_instruction_name(),
                            ins=[], outs=[], engine=inst.engine,
                            sync_info=mybir.SyncInfo(
                                on_wait=extra[i : i + limit], on_update=[]
                            ),
                        )
                        nc.register_instruction(nop)
                        out.append(nop)
                    inst.sync_info = mybir.SyncInfo(
                        on_wait=keep, on_update=list(si.on_update or [])
                    )
                    changed = True
                out.append(inst)
            if changed:
                bb.instructions = out
# ---------------------------------------------------------------------------


def _build_nc(slots, chunks, total_l):
    """Build the SPMD bass module. `slots[i]` holds the geometry of the
    request processed in slot i (descending pairs); `chunks` is the list of
    (offset, length) windows of the packed KV stream; `total_l` its length.
    The `ident` input is a permutation matrix mapping qkv rows (original b)
    to slot columns; the host unpermutes the output rows."""
    nc = bass.Bass()

    def param(name, shape, dt):
        return nc.declare_dram_parameter(name, list(shape), dt, isOutput=False)

    hT = param("hT", [128, KT, B], BF)
    wpp = param("wpp", [128, KT * 3 * HL * D], BF)
    wo = param("wo", [HL, 128, HID], BF)
    kvs = param("kvs", [128, total_l], BF)
    cs = param("cs", [B, 4 * HL * D], F32)
    zmaskp = param("zmask", [128, B], F32)           # 1 iff row s < hist (tail pair)
    identp = param("ident", [B, B], BF)              # permutation matrix
    out_part = nc.declare_dram_parameter("out_part", [B, HID], BF, isOutput=True)

    HD = HL * D  # 512 local attention dims

    with tile.TileContext(nc) as tc:
        with (
            tc.tile_pool(name="const", bufs=1) as cpool,
            tc.tile_pool(name="work", bufs=1) as wpool,
            tc.tile_pool(name="wtiles", bufs=3) as wtp,
            tc.tile_pool(name="wop", bufs=4) as wop,
            tc.tile_pool(name="kv", bufs=6) as kvp,
            tc.tile_pool(name="small", bufs=3) as smp,
        ):
            # ---- constants ----
            ident = cpool.tile([B, B], BF)
            nc.sync.dma_start(out=ident[:], in_=identp[:])
            ones = cpool.tile([128, 1], BF)
            nc.vector.memset(ones[:], 1.0)
            onesb = cpool.tile([1, HL * B], BF)
            nc.vector.memset(onesb[:], 1.0)
            zmask = cpool.tile([128, B], F32)
            nc.sync.dma_start(out=zmask[:], in_=zmaskp[:])
            cs_sb = cpool.tile([B, 4 * HD], F32)
            nc.sync.dma_start(out=cs_sb[:], in_=cs[:])
            hT_sb = cpool.tile([128, KT, B], BF)
            nc.sync.dma_start(out=hT_sb[:], in_=hT[:])

            # packed-KV chunk loads (one DMA each, huge per-partition lines)
            kv_tiles = {}

            def load_chunk(c):
                off, ln = chunks[c]
                kvt = kvp.tile([128, CP_MAX], BF, tag="kv")
                nc.sync.dma_start(out=kvt[:, 0:ln], in_=kvs[:, off : off + ln])
                kv_tiles[c] = kvt

            # accumulators written per-slot, read by the epilogue halves
            atsb = wpool.tile([128, HL * B], F32)   # cached attn, col h*32+i
            nc.vector.memset(atsb[:], 0.0)
            dnm = wpool.tile([1, HL * B], F32)      # cached denom, col h*32+i
            nc.vector.memset(dnm[:], 0.0)

            wo_tiles = {}

            def issue_wo(i):
                wot = wop.tile([128, HID], BF, tag="wot")
                nc.sync.dma_start(out=wot[:], in_=wo[i])
                wo_tiles[i] = wot

            with tc.tile_pool(name="psA", bufs=1, space="PSUM") as psA:
                # PE warmup transpose so `ident` is observed by PE before the
                # real (fp32, single-wait-slot) transposes below.
                tp0 = psA.tile([B, B], BF, tag="tp0")
                nc.tensor.transpose(tp0[:], ident[:], ident[:])

                # ---- phase 1: qkv = hidden @ W_pack (bf16, 8 big weight DMAs) ----
                qkv_ps = psA.tile([B, 3 * HD], F32, tag="qkv")
                for wc in range(8):
                    wpt = wtp.tile([128, 4 * 3 * HD], BF, tag="wpt")
                    nc.sync.dma_start(
                        out=wpt[:],
                        in_=wpp[:, wc * 4 * 3 * HD : (wc + 1) * 4 * 3 * HD],
                    )
                    for j in range(4):
                        kt = wc * 4 + j
                        for n in range(3):
                            nc.tensor.matmul(
                                qkv_ps[:, n * HD : (n + 1) * HD],
                                hT_sb[:, kt, :],
                                wpt[:, j * 3 * HD + n * HD : j * 3 * HD + (n + 1) * HD],
                                start=(kt == 0),
                                stop=(kt == KT - 1),
                            )

                # KV chunk preloads + o_proj weights queue behind the wp
                # stream on the sync DMA queue, in consumption order.
                for c in range(min(6, len(chunks))):
                    load_chunk(c)
                for i in range(HL):
                    issue_wo(i)

                # ---- phase 2: rotary (fp32 DVE, reading PSUM directly),
                # rotated q/k written straight to the bf16 staging tile ----
                qkv_bf = wpool.tile([B, 3 * HD], BF)
                nc.scalar.copy(qkv_bf[:, 2 * HD :], qkv_ps[:, 2 * HD :])

                def rope(src_off, cs_off):
                    src = qkv_ps[:, src_off : src_off + HD]
                    t1 = wpool.tile([B, HD], F32, tag="rope_t1")
                    nc.vector.tensor_tensor(
                        t1[:], src, cs_sb[:, cs_off : cs_off + HD], MUL
                    )
                    sh = wpool.tile([B, HD], F32, tag="rope_sh")
                    sh4 = sh[:].rearrange("b (h d) -> b h d", h=HL)
                    sr4 = src.rearrange("b (h d) -> b h d", h=HL)
                    nc.scalar.copy(sh4[:, :, 0:64], sr4[:, :, 64:128])
                    nc.scalar.copy(sh4[:, :, 64:128], sr4[:, :, 0:64])
                    nc.vector.tensor_tensor(
                        sh[:], sh[:], cs_sb[:, cs_off + HD : cs_off + 2 * HD], MUL
                    )
                    nc.vector.tensor_tensor(
                        qkv_bf[:, src_off : src_off + HD], t1[:], sh[:], ADD
                    )

                rope(0, 0)
                rope(HD, 2 * HD)

            # PE transposes (bf16, permuted to slot order by `ident`)
            qT_bf = wpool.tile([128, HL * B], BF)
            vT = wpool.tile([128, HL * B], F32)
            prod = wpool.tile([128, HL * B], BF)
            e_new = wpool.tile([1, HL * B], F32)
            vTe = wpool.tile([128, HL * B], F32)    # e_new * v_new, broadcast
            with tc.tile_pool(name="psT", bufs=2, space="PSUM") as psT:
                for h in range(HL):
                    tpq = psT.tile([128, B], BF, tag="tpq")
                    nc.tensor.transpose(
                        tpq[:], qkv_bf[:, h * D : (h + 1) * D], ident[:]
                    )
                    tpk = psT.tile([128, B], BF, tag="tpk")
                    nc.tensor.transpose(
                        tpk[:], qkv_bf[:, HD + h * D : HD + (h + 1) * D], ident[:]
                    )
                    tpv = psT.tile([128, B], BF, tag="tpv")
                    nc.tensor.transpose(
                        tpv[:], qkv_bf[:, 2 * HD + h * D : 2 * HD + (h + 1) * D],
                        ident[:],
                    )
                    nc.vector.tensor_copy(qT_bf[:, h * B : (h + 1) * B], tpq[:])
                    nc.scalar.copy(vT[:, h * B : (h + 1) * B], tpv[:])
                    # new-token score terms: q_d * k_d (slot order), bf16
                    nc.vector.tensor_tensor(
                        prod[:, h * B : (h + 1) * B],
                        qT_bf[:, h * B : (h + 1) * B], tpk[:], MUL
                    )

                sn_ps = psT.tile([1, HL * B], F32, tag="sn", bufs=1)
                nc.tensor.matmul(sn_ps[:], ones[:], prod[:], start=True, stop=True)
                nc.scalar.activation(e_new[:], sn_ps[:], EXP_FN)

                # broadcast e_new over partitions once; fold into vT
                e_bfx = wpool.tile([1, HL * B], BF)
                nc.scalar.copy(e_bfx[:], e_new[:])
                ebp = psT.tile([128, HL * B], F32, tag="ebp", bufs=1)
                nc.tensor.matmul(ebp[:], onesb[:], e_bfx[:], start=True, stop=True)
                nc.vector.tensor_tensor(vTe[:], vT[:], ebp[:], MUL)

            dtot = wpool.tile([1, HL * B], F32)
            rec = wpool.tile([1, HL * B], F32)
            nc.vector.memset(rec[:], 1.0)

            atsb_v = atsb[:].rearrange("d (h b2) -> d h b2", h=HL)
            vTe_v = vTe[:].rearrange("d (h b2) -> d h b2", h=HL)
            dnm_v = dnm[:].rearrange("o (h b2) -> o h b2", h=HL)
            e_v = e_new[:].rearrange("o (h b2) -> o h b2", h=HL)
            dtot_v = dtot[:].rearrange("o (h b2) -> o h b2", h=HL)
            rec_v = rec[:].rearrange("o (h b2) -> o h b2", h=HL)

            def ep_math_full():
                # attn = (atsb + e_new*vT) * rec, all 32 slot columns
                r_bf = wpool.tile([1, HL * B], BF)
                nc.vector.tensor_copy(r_bf[:], rec[:])
                att = wpool.tile([128, HL * B], F32)
                with tc.tile_pool(name="psE", bufs=1, space="PSUM") as psE:
                    rbp = psE.tile([128, HL * B], F32, tag="rbp")
                    nc.tensor.matmul(rbp[:], onesb[:], r_bf[:], start=True, stop=True)
                    nc.vector.tensor_tensor(att[:], atsb[:], vTe[:], ADD)
                    nc.vector.tensor_tensor(att[:], att[:], rbp[:], MUL)
                at_bf = wpool.tile([128, HL * B], BF)
                nc.vector.tensor_copy(at_bf[:], att[:])
                return at_bf

            # ---- phase 3: per-request paged attention (slot order) ----
            # Software-pipelined one request ahead: scores(i+1) is issued
            # before attnV(i) so the PE never stalls on the mask->exp->cast
            # round trip through DVE/ACT.
            ph_tiles = {}
            with (
                tc.tile_pool(name="psB", bufs=3, space="PSUM") as psB,
                tc.tile_pool(name="psB2", bufs=2, space="PSUM") as psB2,
            ):
                def ensure_chunk(i):
                    c = slots[i]["chunk"]
                    if c not in kv_tiles:
                        load_chunk(c)

                def do_scores(i):
                    m = slots[i]
                    pb = m["pb"]
                    if pb == 0:
                        return
                    kvt = kv_tiles[m["chunk"]]
                    rel, hist = m["rel"], m["hist"]
                    # scores^T: [128(s), (h, pair)]
                    scp = psB.tile([128, HL, pb], F32, tag="scp")
                    for h in range(HL):
                        qh = qT_bf[:, h * B + i : h * B + i + 1]
                        base = rel + h * hist
                        for p in range(pb):
                            nc.tensor.matmul(
                                scp[:, h, p : p + 1],
                                kvt[:, base + p * 128 : base + p * 128 + 128],
                                qh, start=True, stop=True,
                            )
                    # probs = exp(scores) in bf16 straight off PSUM; rows
                    # >= hist in the tail pair are zeroed by an ACT copy with
                    # a per-partition 0/1 scale (same engine, no extra hop)
                    ph = smp.tile([128, HL, pb], BF, tag="ph", bufs=9)
                    nc.scalar.activation(ph[:], scp[:], EXP_FN)
                    if m["rtail"] < 128:
                        nc.scalar.activation(
                            ph[:, :, pb - 1], ph[:, :, pb - 1], COPY_FN,
                            scale=zmask[:, i : i + 1],
                        )
                    ph_tiles[i] = ph

                def do_attnv(i):
                    m = slots[i]
                    pb = m["pb"]
                    if pb > 0:
                        kvt = kv_tiles[m["chunk"]]
                        ph = ph_tiles.pop(i)
                        vbase = m["rel"] + HL * m["hist"]
                        # attn^T[d, h] = sum_s p[s] * V[s, d]
                        atp = psB.tile([128, HL], F32, tag="atp")
                        for h in range(HL):
                            for p in range(pb):
                                o = vbase + p * HD + h * D
                                nc.tensor.matmul(
                                    atp[:, h : h + 1], kvt[:, o : o + D],
                                    ph[:, h, p : p + 1],
                                    start=(p == 0), stop=(p == pb - 1),
                                )
                        # on vector, NOT scalar: a scalar-queue copy waiting on
                        # attnv(i) would make exp(i+1) wait behind it, coupling
                        # the in-order PE and ACT queues into a per-slot
                        # round-trip chain that paces the whole stream.
                        nc.vector.tensor_copy(atsb_v[:, :, i], atp[:])
                        # denominators: column sums of probs
                        dsp = psB2.tile([1, HL * pb], F32, tag="dsp")
                        nc.tensor.matmul(
                            dsp[:], ones[:],
                            ph[:].rearrange("s h p -> s (h p)"),
                            start=True, stop=True,
                        )
                        nc.vector.reduce_sum(
                            dnm_v[:, :, i],
                            dsp[:].rearrange("o (h p) -> o h p", h=HL),
                            axis=mybir.AxisListType.X,
                        )
                    # incremental 1/(denom + e_new) for this slot
                    nc.vector.tensor_tensor(
                        dtot_v[:, :, i], dnm_v[:, :, i], e_v[:, :, i], ADD
                    )
                    nc.vector.reciprocal(rec_v[:, :, i], dtot_v[:, :, i])

                # scores run LAG slots ahead of attnV: by the time attnv(i)
                # reaches the in-order PE queue, exp(i) is long finished, so
                # the PE never blocks on the ACT round trip mid-stream.
                LAG = 6
                for j in range(LAG):
                    ensure_chunk(j)
                    do_scores(j)
                for i in range(B):
                    do_attnv(i)
                    if i + LAG < B:
                        ensure_chunk(i + LAG)
                        do_scores(i + LAG)

            # ---- epilogue + output projection (after loop pools close) ----
            at_bf = ep_math_full()
            outf = wpool.tile([B, HID], BF)
            with tc.tile_pool(name="psC", bufs=3, space="PSUM") as psC:
                for n in range(8):
                    opsn = psC.tile([B, HD], F32, tag="ops")
                    for h in range(HL):
                        nc.tensor.matmul(
                            opsn[:],
                            at_bf[:, h * B : (h + 1) * B],
                            wo_tiles[h][:, n * HD : (n + 1) * HD],
                            start=(h == 0),
                            stop=(h == HL - 1),
                        )
                    if n % 2:
                        nc.scalar.copy(outf[:, n * HD : (n + 1) * HD], opsn[:])
                    else:
                        nc.vector.tensor_copy(outf[:, n * HD : (n + 1) * HD], opsn[:])
            nc.gpsimd.dma_start(out=out_part[:], in_=outf[:])

    _split_excess_waits(nc)
    return nc


def _host_prep(hidden, W_pack, o_proj_weight, k_cache, v_cache, hist, block_offsets):
    """Build the 8 per-core input maps (numpy only)."""
    hidden = np.asarray(hidden, np.float32)
    W_pack = np.asarray(W_pack, np.float32)
    o_proj_weight = np.asarray(o_proj_weight, np.float32)
    k_cache = np.asarray(k_cache, np.float32)
    v_cache = np.asarray(v_cache, np.float32)
    hist = np.asarray(hist, np.int64)
    block_offsets = np.asarray(block_offsets, np.int64)

    pairs = [int((h + 127) // 128) for h in hist]
    # slot order: descending pairs so the first epilogue half (slots 0..15)
    # triggers while plenty of KV stream remains to overlap its o_proj
    order = sorted(range(B), key=lambda b: (-pairs[b], b))

    # rope tables, scale folded into the q tables
    inv_freq = 1.0 / (ROPE_BASE ** (np.arange(0, D, 2, dtype=np.float32) / D))
    ang = hist.astype(np.float32)[:, None] * inv_freq[None, :]        # [B, 64]
    cos128 = np.concatenate([np.cos(ang), np.cos(ang)], -1)           # [B, 128]
    sin128 = np.concatenate([np.sin(ang), np.sin(ang)], -1)
    sign = np.concatenate([-np.ones(64), np.ones(64)]).astype(np.float32)
    sc = 1.0 / math.sqrt(D)
    tile_h = lambda x: np.tile(x, (1, HL)).astype(np.float32)         # [B, 512]
    cs = np.concatenate(
        [tile_h(cos128 * sc), tile_h(sin128 * sign * sc),
         tile_h(cos128), tile_h(sin128 * sign)], -1,
    )                                                                 # [B, 2048]

    # per-slot geometry + packed-stream layout
    hs = [int(hist[order[i]]) for i in range(B)]
    pbs = [pairs[order[i]] for i in range(B)]
    rtail = [hs[i] - 128 * (pbs[i] - 1) if pbs[i] > 0 else 128 for i in range(B)]
    Ls = [HL * hs[i] + pbs[i] * HL * D for i in range(B)]

    chunks = []
    chunk_of_slot = []
    G = []
    cur_off = cur_len = 0
    for i in range(B):
        if cur_len + Ls[i] > CP_MAX:
            chunks.append((cur_off, cur_len))
            cur_off += cur_len
            cur_len = 0
        G.append(cur_off + cur_len)
        chunk_of_slot.append(len(chunks))
        cur_len += Ls[i]
    chunks.append((cur_off, cur_len))
    total_l = cur_off + cur_len
    slots = [
        dict(hist=hs[i], pb=pbs[i], rtail=rtail[i],
             rel=G[i] - chunks[chunk_of_slot[i]][0], chunk=chunk_of_slot[i])
        for i in range(B)
    ]

    zmask = np.zeros((128, B), dtype=np.float32)
    for i, r in enumerate(rtail):
        zmask[:r, i] = 1.0

    hT = np.ascontiguousarray(hidden.T)                               # [4096, 32]
    hT_bf = np.ascontiguousarray(
        hT.astype(BF_NP).reshape(KT, 128, B).transpose(1, 0, 2)
    )                                                                 # [128, KT, B]

    # gather caches via the block table (b-major)
    kf = k_cache[block_offsets.reshape(-1)].reshape(B, MAX_KV, H, D)
    vf = v_cache[block_offsets.reshape(-1)].reshape(B, MAX_KV, H, D)

    # permutation matrix: column slot i picks original request order[i]
    ident = np.zeros((B, B), dtype=BF_NP)
    ident[np.asarray(order), np.arange(B)] = 1.0

    in_maps = []
    for c in range(NCORES):
        h0 = c * HL
        qcols = np.arange(h0 * D, (h0 + HL) * D)
        wp_c = np.concatenate(
            [W_pack[:, qcols], W_pack[:, HID + qcols], W_pack[:, 2 * HID + qcols]],
            axis=1,
        )                                                             # [4096, 1536]
        wp_bf = np.ascontiguousarray(
            wp_c.astype(BF_NP).reshape(KT, 128, 3 * HL * D)
            .transpose(1, 0, 2).reshape(128, KT * 3 * HL * D)
        )                                                             # [128, KT*1536]

        wo_c = np.ascontiguousarray(o_proj_weight[:, qcols].T)        # [512, 4096]
        wo_bf = wo_c.astype(BF_NP).reshape(HL, 128, HID)

        kvs = np.zeros((128, total_l), dtype=BF_NP)
        for i in range(B):
            b = order[i]
            hb, pb, g = hs[i], pbs[i], G[i]
            if pb == 0:
                continue
            kb = kf[b, :hb, h0 : h0 + HL, :]                          # [hb, 4, 128]
            kvs[:, g : g + HL * hb] = (
                kb.transpose(2, 1, 0).reshape(D, HL * hb).astype(BF_NP)
            )
            vb = np.zeros((pb * 128, HL, D), np.float32)
            vb[:hb] = vf[b, :hb, h0 : h0 + HL, :]
            g4 = g + HL * hb
            kvs[:, g4 : g4 + pb * HL * D] = (
                vb.reshape(pb, 128, HL * D).transpose(1, 0, 2)
                .reshape(128, pb * HL * D).astype(BF_NP)
            )

        in_maps.append({
            "hT": hT_bf, "wpp": wp_bf, "wo": wo_bf,
            "kvs": kvs,
            "cs": cs, "zmask": zmask, "ident": ident,
        })
    return slots, chunks, total_l, order, in_maps


def kernel(hidden_states, W_pack, o_proj_weight, k_cache, v_cache,
           history_lengths, block_offsets):
    global LAST_RESULTS
    slots, chunks, total_l, order, in_maps = _host_prep(
        hidden_states, W_pack, o_proj_weight, k_cache, v_cache,
        history_lengths, block_offsets,
    )
    nc = _build_nc(slots, chunks, total_l)
    trace = bool(int(os.environ.get("KERNEL_TRACE", "0")))
    res = run_bass_kernel_spmd(nc, in_maps, list(range(NCORES)), trace=trace)
    LAST_RESULTS = res
    acc = np.zeros((B, HID), np.float32)
    for c in range(NCORES):
        acc += res.results[c]["out_part"].astype(np.float32)
    out = np.zeros((B, HID), np.float32)
    out[np.asarray(order)] = acc                   # slot rows -> original rows
    return out
